# revision 29
# baseline (speedup 1.0000x reference)
"""Bass/Tile TRN2 kernel: adaptive min 2D pooling (8x8 grid) of [B,512,512] f32.

Full input [128, 512, 512] f32 -> output [128, 64] f32.
Data parallel over 8 NeuronCores: 16 matrices per core. The kernel is
DMA-bandwidth-bound (16.78 MB HBM reads per core at 360 GB/s ~= 46.6 us
streaming floor); the design minimizes everything outside the stream.

"band" layout (default): partition p <-> (matrix m = p//8, row-band
b = p%8).  Each partition's SBUF line is one 128KB contiguous HBM run
holding exactly the 64x512 region that produces that partition's 8
outputs, so the pooling min NEVER crosses partitions:

  - input streams as 13 full chunks (4 rows/partition, 8KB contiguous
    descriptors, 2913ns each at the 360 GB/s roofline) + 12 single-row
    tail slices (2KB descriptors, 728ns) alternating the two HWDGE
    rings (sync/scalar).  The tail slices keep the last DVE reduce
    small (594ns vs 2194ns) and let the DVE backlog drain just-in-time
    with the stream (DVE needs 2194ns/chunk vs DMA 2913ns).
  - one DVE reduce per chunk (axis XY over rows+cols within each col
    group) -> vv[:, k*8:(k+1)*8]; one final strided merge reduce
    vv[128, 200] -> v[128, 8].
  - single output DMA v[128,8] -> y (contiguous [128,8] view of y).
  - no PE transpose / PSUM / identity matrix / Pool or Activation
    compute anywhere: the cross-partition stage of the old "quad"
    layout (transpose -> reduce -> transpose -> copy -> SWDGE DMA,
    ~3.5us of tail chain) is gone entirely.

The old "quad" layout (partition p = 4 consecutive rows of one matrix,
PE-transpose cross-partition stage) is kept for A/B benchmarking.
"""

import threading

import numpy as np

B, N, M = 128, 512, 512
GRID = 8
NCORES = 8
BL = B // NCORES  # 16 matrices per core

_lock = threading.RLock()
_cache: dict = {}


def _build(
    n_iters: int = 1,
    layout: str = "band",
    n_full: int = 13,
    chunk_bufs: int = 10,
    out_ring: str = "sync",
    slice_wait_ns: int = 35000,
    slice_step_ns: int = 700,
    split_chunk: int = 1,
    n_half_rows: int = 3,
    n_quarter_rows: int = 0,
    n_rings: int = 2,
    premerge2: int = 0,
):
    import concourse.bacc as bacc
    import concourse.mybir as mybir
    import concourse.tile as tile

    f32 = mybir.dt.float32

    nc = bacc.Bacc("TRN2", target_bir_lowering=False, debug=False)
    x = nc.dram_tensor("x", [BL, N, M], f32, kind="ExternalInput").ap()
    y = nc.dram_tensor("y", [BL, GRID * GRID], f32, kind="ExternalOutput").ap()

    with tile.TileContext(nc) as tc:
        if layout == "band":
            with (
                tc.tile_pool(name="inp", bufs=chunk_bufs) as inp,
                tc.tile_pool(name="vvp", bufs=2) as vvp,
                tc.tile_pool(name="vp", bufs=2) as vp,
            ):
                for _ in range(n_iters):  # n_iters>1 only for benchmarking
                    _emit_band_iter(
                        nc, mybir, tc, x, y, inp, vvp, vp, n_full, out_ring,
                        # Manual scheduler waits only make sense for the
                        # single-shot build; slope-timing builds (n_iters>1)
                        # rely on steady-state backpressure instead.
                        slice_wait_ns if n_iters == 1 else None,
                        slice_step_ns,
                        split_chunk=bool(split_chunk),
                        n_half_rows=n_half_rows,
                        n_quarter_rows=n_quarter_rows,
                        n_rings=n_rings,
                        premerge2=bool(premerge2),
                    )
        else:
            raise ValueError(f"unknown layout {layout!r}")

    nc.compile()
    return nc


def _emit_band_iter(
    nc, mybir, tc, x, y, inp, vvp, vp, n_full, out_ring,
    slice_wait_ns, slice_step_ns, split_chunk=True, n_half_rows=3,
    n_quarter_rows=0, n_rings=2, premerge2=True,
):
    """One pooling iteration in the (matrix, band) partition layout.

    The stream is a list of (row, n_rows, col, n_cols, slot) segments over
    each partition's 64x512 band; one DMA + one DVE min-reduce per segment,
    reduced into vv slot `slot` (half-row segments share a slot, writing
    disjoint g-columns).  Segment sizing shapes the tail: big 4-row chunks
    up front, then (optionally) 2-row half-chunks, single-row slices and
    half-row slices so the last transfer's dependent reduce is tiny and the
    DVE backlog drains just-in-time with the stream.
    """
    f32 = mybir.dt.float32
    rings = [nc.sync, nc.scalar][:n_rings]
    # [BL, 512, 512] -> [(m b) = 128, 32768]: partition line = one 64-row
    # band = 128KB contiguous HBM ((m b) folds because b stride 128KB x 8
    # = m stride 1MB).
    xv = x.rearrange("m (b rr) c -> (m b) (rr c)", b=GRID)

    segs = []  # (row, n_rows, col, n_cols, slot)
    slot = 0
    n_chunks = n_full if not split_chunk else n_full - 1
    for cr in range(n_chunks):
        segs.append((cr * 4, 4, 0, 512, slot))
        slot += 1
    if split_chunk:  # last chunk as two 2-row halves (earlier sems, smaller
        for h in range(2):  # reduces: gives the tail DVE backlog slack)
            segs.append(((n_full - 1) * 4 + h * 2, 2, 0, 512, slot))
            slot += 1
    n_chunk_segs = len(segs)
    merge_slot = slot  # chunk pre-merge target
    slot += 1
    first_slice = n_full * 4
    n_rows_left = 64 - first_slice
    for j in range(n_rows_left - n_half_rows - n_quarter_rows):
        segs.append((first_slice + j, 1, 0, 512, slot))
        slot += 1
    premerge2_slot = None
    last_full_seg = len(segs) - 1
    if premerge2 and slot > merge_slot + 1:
        # Reserve a slot for a second pre-merge over [merge_slot..here), so
        # the final merge only spans the premerge2 slot + half/quarter rows.
        premerge2_slot = slot
        slot += 1
    for j in range(n_rows_left - n_half_rows - n_quarter_rows,
                   n_rows_left - n_quarter_rows):
        # tail rows as two half-rows sharing one slot: sems arrive every
        # 364ns vs ~327ns reduce, so the DVE drains just-in-time
        row = first_slice + j
        segs.append((row, 1, 0, 256, slot))
        segs.append((row, 1, 256, 256, slot))
        slot += 1
    for j in range(n_rows_left - n_quarter_rows, n_rows_left):
        # final rows as four quarter-rows: the very last reduce is ~194ns
        row = first_slice + j
        for q in range(4):
            segs.append((row, 1, q * 128, 128, slot))
        slot += 1

    vv = vvp.tile([128, 8 * slot], f32)

    for di, (row, n_rows, col, n_cols, sl) in enumerate(segs):
        t = inp.tile([128, 2048], f32)
        src = xv[:, row * 512 + col : row * 512 + col + (n_rows - 1) * 512 + n_cols]
        eng = rings[di % len(rings)]
        j = di - n_chunk_segs
        if j >= 0 and slice_wait_ns is not None:
            # Tail slices: the compile-time Tile scheduler would happily
            # hoist these tiny DMAs ahead of full chunks (pushing 2913ns
            # chunks + their 2194ns reduces into the stream tail), so for
            # the single-shot build pin each one behind the chunk stream
            # with a manual scheduler wait (tile_wait_until, ns timebase).
            with tc.tile_wait_until((slice_wait_ns + j * slice_step_ns) / 1e6):
                eng.dma_start(t[:, : n_rows * n_cols], src)
        else:
            eng.dma_start(t[:, : n_rows * n_cols], src)
        goff = col // 64
        gn = n_cols // 64
        out = vv[:, sl * 8 + goff : sl * 8 + goff + gn]
        if n_rows > 1:
            nc.vector.tensor_reduce(
                out,
                t[:, : n_rows * n_cols].rearrange(
                    "p (r g gc) -> p g r gc", g=gn, gc=64
                ),
                axis=mybir.AxisListType.XY,
                op=mybir.AluOpType.min,
            )
        else:
            nc.vector.tensor_reduce(
                out,
                t[:, :n_cols].rearrange("p (g gc) -> p g gc", gc=64),
                axis=mybir.AxisListType.X,
                op=mybir.AluOpType.min,
            )
        if di == n_chunk_segs - 1:
            # Pre-merge all chunk slots while the tail slices stream; the
            # final merge then spans only the remaining slots.
            nc.vector.tensor_reduce(
                vv[:, merge_slot * 8 : (merge_slot + 1) * 8],
                vv[:, : merge_slot * 8].rearrange("p (k g) -> p g k", g=GRID),
                axis=mybir.AxisListType.X,
                op=mybir.AluOpType.min,
            )
        if premerge2_slot is not None and di == last_full_seg:
            nc.vector.tensor_reduce(
                vv[:, premerge2_slot * 8 : (premerge2_slot + 1) * 8],
                vv[:, merge_slot * 8 : premerge2_slot * 8].rearrange(
                    "p (k g) -> p g k", g=GRID
                ),
                axis=mybir.AxisListType.X,
                op=mybir.AluOpType.min,
            )

    final_slot = merge_slot if premerge2_slot is None else premerge2_slot
    v = vp.tile([128, 8], f32)
    nc.vector.tensor_reduce(
        v[:],
        vv[:, final_slot * 8 :].rearrange("p (k g) -> p g k", g=GRID),
        axis=mybir.AxisListType.X,
        op=mybir.AluOpType.min,
    )
    out_eng = {"sync": nc.sync, "scalar": nc.scalar, "gpsimd": nc.gpsimd}[
        out_ring
    ]
    # y [16, 64] viewed [(m b) = 128, 8] is contiguous; 32B runs/partition
    out_eng.dma_start(y.rearrange("m (b g) -> (m b) g", b=GRID), v[:])


def _get_nc():
    with _lock:
        if "nc" not in _cache:
            _cache["nc"] = _build()
        return _cache["nc"]


def _get_runner():
    """Build the shard_map-jitted 8-core runner ONCE and reuse it across
    kernel() calls (run_bass_kernel_spmd re-jits per call, ~seconds of host
    overhead). Mirrors bass2jax.run_bass_via_pjrt's multi-core wiring."""
    if "runner" in _cache:
        return _cache["runner"]

    import jax
    from jax.sharding import Mesh, PartitionSpec
    from jax.experimental.shard_map import shard_map

    from concourse import bass2jax, mybir

    nc = _get_nc()
    bass2jax.install_neuronx_cc_hook()

    partition_name = nc.partition_id_tensor.name if nc.partition_id_tensor else None
    in_names, out_names, out_avals = [], [], []
    for alloc in nc.m.functions[0].allocations:
        if not isinstance(alloc, mybir.MemoryLocationSet):
            continue
        name = alloc.memorylocations[0].name
        if alloc.kind == "ExternalInput":
            if name != partition_name:
                in_names.append(name)
        elif alloc.kind == "ExternalOutput":
            out_names.append(name)
            out_avals.append(
                jax.core.ShapedArray(
                    tuple(alloc.tensor_shape), mybir.dt.np(alloc.dtype)
                )
            )
    assert in_names == ["x"] and out_names == ["y"]
    all_in_names = list(in_names) + list(out_names)
    if partition_name is not None:
        all_in_names.append(partition_name)

    def _body(*args):
        operands = list(args)
        if partition_name is not None:
            operands.append(bass2jax.partition_id_tensor())
        outs = bass2jax._bass_exec_p.bind(
            *operands,
            out_avals=tuple(out_avals),
            in_names=tuple(all_in_names),
            out_names=tuple(out_names),
            lowering_input_output_aliases=(),
            sim_require_finite=True,
            sim_require_nnan=True,
            nc=nc,
        )
        return tuple(outs)

    devices = jax.devices()[:NCORES]
    mesh = Mesh(np.asarray(devices), ("core",))
    sharded = jax.jit(
        shard_map(
            _body,
            mesh=mesh,
            in_specs=(PartitionSpec("core"),) * 2,
            out_specs=(PartitionSpec("core"),),
            check_rep=False,
        ),
        donate_argnums=(1,),
        keep_unused=True,
    )
    _cache["runner"] = sharded
    return sharded


def _kernel_fallback(xs: np.ndarray) -> np.ndarray:
    from concourse.bass_utils import run_bass_kernel_spmd

    nc = _get_nc()
    in_maps = [{"x": xs[i * BL : (i + 1) * BL]} for i in range(NCORES)]
    r = run_bass_kernel_spmd(nc, in_maps, list(range(NCORES)))
    return np.concatenate([r.results[i]["y"] for i in range(NCORES)], axis=0)


def kernel(sim_matrices: np.ndarray) -> np.ndarray:
    xs = np.ascontiguousarray(sim_matrices, dtype=np.float32)
    assert xs.shape == (B, N, M), xs.shape
    with _lock:
        try:
            runner = _get_runner()
            zeros = np.zeros((B, GRID * GRID), np.float32)
            (y_global,) = runner(xs, zeros)
            return np.asarray(y_global)
        except Exception:
            return _kernel_fallback(xs)
